# revision 6
# baseline (speedup 1.0000x reference)
"""Multi-head attention (bz=2, slen=4096, d=768, 12 heads) on 8 trn2 NeuronCores.

Sharding: 8 cores = 2 (batch) x 2 (head halves of 6) x 2 (q halves of 2048).
Each core computes its q-slice of the partial output for its 6 heads; host sums
the two head-half partials per (batch, q-half) and adds b_o.

Key device-side structure (per core):
  - projections contract input features (768) on partitions; q/k inputs are fed
    pre-transposed [768, L] so qh^T/kh^T come out feature-major (QK operands),
    while vh is produced position-major (PV stationary operand) by using the
    transposed v as the stationary operand instead.
  - mask handling is exact and free: masked k-positions are gathered away on the
    host (softmax with -1e9 gives exactly 0 in fp32), and right-padding to a
    multiple of 128 is neutralized by a per-position "valid" column appended to
    v, which simultaneously produces the softmax row-sums during PV.
  - scores stay in S^T layout [k-pos partitions, q free]: softmax needs only one
    ACT pass (exp with fused 1/sqrt(64) scale); row-sums fall out of PV; the
    per-(head,q) normalization is applied to attn^T (64 rows) instead of P.
  - QK packs head pairs into the PE array rows (K=64 x 2 via tile_position).
  - matmuls run in float32r (full fp32 data, fast PE mode).
"""

import os

import numpy as np

_CACHE = {}


def _build(KP):
    import concourse.bass as bass
    import concourse.mybir as mybir
    import concourse.tile as tile
    from concourse import bacc

    F32 = mybir.dt.float32
    F32R = mybir.dt.float32r
    EXP = mybir.ActivationFunctionType.Exp

    F = 768          # model dim
    M = 384          # output features per core (6 heads x 64)
    QL = 2048        # q rows per core
    D = 64           # head dim
    H = 6            # heads per core
    KT = KP // 128   # k tiles
    QCW = 512        # q chunk width
    NQC = QL // QCW

    nc = bacc.Bacc("TRN2", target_bir_lowering=False, debug=False, num_devices=8)

    qT_d = nc.dram_tensor("qT", [F, QL], F32R, kind="ExternalInput").ap()
    kT_d = nc.dram_tensor("kT", [F, KP], F32R, kind="ExternalInput").ap()
    vT_d = nc.dram_tensor("vT", [F, KP], F32R, kind="ExternalInput").ap()
    wq_d = nc.dram_tensor("wq", [F, M], F32R, kind="ExternalInput").ap()
    wk_d = nc.dram_tensor("wk", [F, M], F32R, kind="ExternalInput").ap()
    wv_d = nc.dram_tensor("wv", [F, M], F32R, kind="ExternalInput").ap()
    wo_d = nc.dram_tensor("wo", [M, F], F32R, kind="ExternalInput").ap()
    valid_d = nc.dram_tensor("valid", [128, KT, 1], F32, kind="ExternalInput").ap()
    out_d = nc.dram_tensor("out", [QL, F], F32, kind="ExternalOutput").ap()

    with tile.TileContext(nc) as tc:
        with (
            tc.tile_pool(name="weights", bufs=1) as wp,
            tc.tile_pool(name="acts", bufs=1) as ap_,
            tc.tile_pool(name="ptp", bufs=2) as ptp,
            tc.tile_pool(name="small", bufs=2) as sp,
            tc.tile_pool(name="atp", bufs=2) as atp,
            tc.tile_pool(name="obp", bufs=2) as obp,
        ):
            # ---- resident weights / tables ----
            wq_s = wp.tile([128, 6, M], F32R)
            nc.sync.dma_start(out=wq_s, in_=wq_d.rearrange("(t p) m -> p t m", p=128))
            wk_s = wp.tile([128, 6, M], F32R)
            nc.sync.dma_start(out=wk_s, in_=wk_d.rearrange("(t p) m -> p t m", p=128))
            wv_s = wp.tile([128, 6, M], F32R)
            nc.sync.dma_start(out=wv_s, in_=wv_d.rearrange("(t p) m -> p t m", p=128))
            wo_s = wp.tile([64, H, F], F32R)
            nc.sync.dma_start(out=wo_s, in_=wo_d.rearrange("(h p) n -> p h n", p=64))
            valid_s = wp.tile([128, KT, 1], F32)
            nc.sync.dma_start(out=valid_s, in_=valid_d)

            # ---- resident projected activations ----
            qhT = ap_.tile([128, 3, QL], F32R)     # feature-major, head pair P
            khT = ap_.tile([128, 3, KP], F32R)
            vh = ap_.tile([128, KT, H, D + 1], F32R)  # position-major + valid col

            # ---- stage A: projections ----
            with (
                tc.tile_pool(name="xc", bufs=2) as xcp,
                tc.tile_pool(name="pp", bufs=3, space="PSUM") as ppp,
            ):
                # v: position-major (stationary = vT tiles, moving = wv)
                for c0 in range(0, KP, QCW):
                    cw = min(QCW, KP - c0)
                    xc = xcp.tile([128, 6, QCW], F32R, tag="xc")
                    nc.sync.dma_start(
                        out=xc[:, :, :cw],
                        in_=vT_d[:, c0:c0 + cw].rearrange("(t p) n -> p t n", p=128),
                    )
                    for t in range(cw // 128):
                        kt = c0 // 128 + t
                        ps = ppp.tile([128, M], F32, tag="ps")
                        for K in range(6):
                            nc.tensor.matmul(
                                ps,
                                lhsT=xc[:, K, t * 128:(t + 1) * 128],
                                rhs=wv_s[:, K, :],
                                start=(K == 0), stop=(K == 5),
                            )
                        nc.vector.tensor_copy(
                            out=vh[:, kt, :, 0:D],
                            in_=ps.rearrange("p (h d) -> p h d", h=H),
                        )
                for h in range(H):
                    nc.vector.tensor_copy(out=vh[:, :, h, D:D + 1], in_=valid_s)

                # k then q: feature-major
                for (xd, w_s, dst, L) in (
                    (kT_d, wk_s, khT, KP),
                    (qT_d, wq_s, qhT, QL),
                ):
                    for c0 in range(0, L, QCW):
                        cw = min(QCW, L - c0)
                        xc = xcp.tile([128, 6, QCW], F32R, tag="xc")
                        nc.sync.dma_start(
                            out=xc[:, :, :cw],
                            in_=xd[:, c0:c0 + cw].rearrange("(t p) n -> p t n", p=128),
                        )
                        for m in range(3):
                            ps = ppp.tile([128, QCW], F32, tag="ps2")
                            for K in range(6):
                                nc.tensor.matmul(
                                    ps[:, :cw],
                                    lhsT=w_s[:, K, m * 128:(m + 1) * 128],
                                    rhs=xc[:, K, :cw],
                                    start=(K == 0), stop=(K == 5),
                                )
                            nc.vector.tensor_copy(out=dst[:, m, c0:c0 + cw],
                                                  in_=ps[:, :cw])

            # ---- stage B: attention + output projection ----
            with (
                tc.tile_pool(name="sps", bufs=2, space="PSUM") as sps,
                tc.tile_pool(name="spa", bufs=1, space="PSUM") as spa,
                tc.tile_pool(name="spo", bufs=2, space="PSUM") as spo,
            ):
                for qc in range(NQC):
                    q0 = qc * QCW
                    at_all = atp.tile([64, H, QCW], F32R, tag="at")
                    for p in range(3):
                        pa = spa.tile([D + 1, QCW], F32, tag="pa")
                        pb = spa.tile([D + 1, QCW], F32, tag="pb")
                        pacc = (pa, pb)

                        def qk(kt, _p=p, _q0=q0):
                            ps = sps.tile([128, 2 * QCW], F32, tag="ps")
                            for j in range(2):
                                nc.tensor.matmul(
                                    ps[:, j * QCW:(j + 1) * QCW],
                                    lhsT=khT[j * 64:(j + 1) * 64, _p,
                                             kt * 128:(kt + 1) * 128],
                                    rhs=qhT[j * 64:(j + 1) * 64, _p,
                                            _q0:_q0 + QCW],
                                    start=True, stop=True,
                                )
                            return ps

                        ps_cur = qk(0)
                        for kt in range(KT):
                            ps_next = qk(kt + 1) if kt + 1 < KT else None
                            pt = ptp.tile([128, 2 * QCW], F32R, tag="pt")
                            nc.scalar.activation(pt, ps_cur, EXP, scale=0.125)
                            for j in range(2):
                                nc.tensor.matmul(
                                    pacc[j],
                                    lhsT=vh[:, kt, 2 * p + j, :],
                                    rhs=pt[:, j * QCW:(j + 1) * QCW],
                                    start=(kt == 0), stop=(kt == KT - 1),
                                )
                            ps_cur = ps_next

                        for j in range(2):
                            h = 2 * p + j
                            rr = sp.tile([1, QCW], F32, tag="rr")
                            nc.vector.reciprocal(rr, pacc[j][D:D + 1, :])
                            bc = sp.tile([64, QCW], F32, tag="bc")
                            nc.gpsimd.partition_broadcast(bc, rr)
                            nc.vector.tensor_mul(at_all[:, h, :], pacc[j][0:D, :], bc)

                    for m2 in range(QCW // 128):
                        ob = obp.tile([128, F], F32, tag="ob")
                        for nch in range(2):
                            po = spo.tile([128, 384], F32, tag="po")
                            for h in range(H):
                                nc.tensor.matmul(
                                    po,
                                    lhsT=at_all[:, h,
                                                m2 * 128:(m2 + 1) * 128],
                                    rhs=wo_s[:, h,
                                             nch * 384:(nch + 1) * 384],
                                    start=(h == 0), stop=(h == H - 1),
                                )
                            nc.vector.tensor_copy(out=ob[:, nch * 384:(nch + 1) * 384],
                                                  in_=po)
                        nc.sync.dma_start(
                            out=out_d[q0 + m2 * 128:q0 + (m2 + 1) * 128, :],
                            in_=ob,
                        )

    nc.compile()
    return nc


last_results = None


def _ensure_ntff_hook():
    """Install the axon NTFF profile hook if the image's antenv lacks it.

    trn_agent_boot intends to register this hook (see trn_boot.py step 6); on
    images whose antenv has no axon_hooks module it degrades. Recreate the
    module so trace=True works; silently no-op if anything is unavailable.
    """
    import sys
    import types
    try:
        import antenv.axon_hooks  # noqa: F401
        return
    except ImportError:
        pass
    try:
        import antenv
        from trn_agent_boot.trn_boot import _ntff_profile_via_ctypes
        hook = _ntff_profile_via_ctypes("/opt/axon/libaxon_pjrt.so")
        mod = types.ModuleType("antenv.axon_hooks")
        mod.get_axon_ntff_profile_hook = lambda: hook
        mod.set_axon_ntff_profile_hook = lambda h: None
        sys.modules["antenv.axon_hooks"] = mod
        antenv.axon_hooks = mod
    except Exception:
        pass


def kernel(v, k, q, mask, w_v, w_k, w_q, w_o, b_o):
    global last_results
    from concourse import bass_utils

    v = np.asarray(v, dtype=np.float32)
    k = np.asarray(k, dtype=np.float32)
    q = np.asarray(q, dtype=np.float32)
    mask = np.asarray(mask)
    w_v = np.asarray(w_v, dtype=np.float32)
    w_k = np.asarray(w_k, dtype=np.float32)
    w_q = np.asarray(w_q, dtype=np.float32)
    w_o = np.asarray(w_o, dtype=np.float32)
    b_o = np.asarray(b_o, dtype=np.float32)

    BZ, SL, F = q.shape
    QL = SL // 2

    kept = [np.flatnonzero(mask[b, 0, 0] == 0) for b in range(BZ)]
    klens = [len(x) for x in kept]
    KP = max(128, -(-max(klens) // 128) * 128)
    KT = KP // 128

    if KP not in _CACHE:
        _CACHE[KP] = _build(KP)
    nc = _CACHE[KP]

    # per-batch gathered/padded transposed k/v and validity tables
    kTs, vTs, valids = [], [], []
    for b in range(BZ):
        kt = np.zeros((F, KP), np.float32)
        vt = np.zeros((F, KP), np.float32)
        kt[:, :klens[b]] = k[b, kept[b]].T
        vt[:, :klens[b]] = v[b, kept[b]].T
        kTs.append(kt)
        vTs.append(vt)
        val = np.zeros(KP, np.float32)
        val[:klens[b]] = 1.0
        valids.append(np.ascontiguousarray(val.reshape(KT, 128).T)[:, :, None])

    in_maps = []
    for c in range(8):
        b, hg, qg = c // 4, (c // 2) % 2, c % 2
        mc = slice(384 * hg, 384 * (hg + 1))
        qr = slice(QL * qg, QL * (qg + 1))
        in_maps.append({
            "qT": np.ascontiguousarray(q[b, qr].T),
            "kT": kTs[b],
            "vT": vTs[b],
            "wq": np.ascontiguousarray(w_q[:, mc]),
            "wk": np.ascontiguousarray(w_k[:, mc]),
            "wv": np.ascontiguousarray(w_v[:, mc]),
            "wo": np.ascontiguousarray(w_o[mc, :]),
            "valid": valids[b],
        })

    trace = bool(os.environ.get("KERNEL_TRACE")) or bool(os.environ.get("BASS_TRACE"))
    if trace:
        _ensure_ntff_hook()
    try:
        res = bass_utils.run_bass_kernel_spmd(
            nc, in_maps, core_ids=list(range(8)),
            trace=trace,
            trace_cores=list(range(8)) if trace else None,
        )
    except Exception:
        if not trace:
            raise
        os.environ["BASS_NEVER_TRACE"] = "1"
        try:
            res = bass_utils.run_bass_kernel_spmd(
                nc, in_maps, core_ids=list(range(8)),
            )
        finally:
            del os.environ["BASS_NEVER_TRACE"]
    last_results = res

    out = np.empty((BZ, SL, F), np.float32)
    for b in range(BZ):
        for qg in range(2):
            c0 = b * 4 + qg          # hg = 0
            c1 = b * 4 + 2 + qg      # hg = 1
            out[b, QL * qg:QL * (qg + 1)] = (
                res.results[c0]["out"] + res.results[c1]["out"] + b_o
            )
    return out


# revision 10
# speedup vs baseline: 1.2456x; 1.2456x over previous
"""Multi-head attention (bz=2, slen=4096, d=768, 12 heads) on 8 trn2 NeuronCores.

Sharding: 8 cores = 2 (batch) x 2 (head halves of 6) x 2 (q halves of 2048).
Each core computes its q-slice of the partial output for its 6 heads; host sums
the two head-half partials per (batch, q-half) and adds b_o.

Key device-side structure (per core):
  - projections contract input features (768) on partitions; q/k inputs are fed
    pre-transposed [768, L] so qh^T/kh^T come out feature-major (QK operands),
    while vh is produced position-major (PV stationary operand) by using the
    transposed v as the stationary operand instead.
  - mask handling is exact and free: masked k-positions are gathered away on the
    host (softmax with -1e9 gives exactly 0 in fp32), and right-padding to a
    multiple of 128 is neutralized by a per-position "valid" column appended to
    v, which simultaneously produces the softmax row-sums during PV.
  - scores stay in S^T layout [k-pos partitions, q free]: softmax needs only one
    ACT pass (exp with fused 1/sqrt(64) scale); row-sums fall out of PV; the
    per-(head,q) normalization is applied to attn^T (64 rows) instead of P.
  - QK packs head pairs into the PE array rows (K=64 x 2 via tile_position).
  - matmuls run in float32r (full fp32 data, fast PE mode).
"""

import os

import numpy as np

_CACHE = {}


def _build(KP):
    import concourse.bass as bass
    import concourse.mybir as mybir
    import concourse.tile as tile
    from concourse import bacc

    F32 = mybir.dt.float32
    F32R = mybir.dt.float32r
    EXP = mybir.ActivationFunctionType.Exp

    F = 768          # model dim
    M = 384          # output features per core (6 heads x 64)
    QL = 2048        # q rows per core
    D = 64           # head dim
    H = 6            # heads per core
    KT = KP // 128   # k tiles
    QCW = 512        # q chunk width
    NQC = QL // QCW

    nc = bacc.Bacc("TRN2", target_bir_lowering=False, debug=False, num_devices=8)

    qT_d = nc.dram_tensor("qT", [F, QL], F32R, kind="ExternalInput").ap()
    kT_d = nc.dram_tensor("kT", [F, KP], F32R, kind="ExternalInput").ap()
    vT_d = nc.dram_tensor("vT", [F, KP], F32R, kind="ExternalInput").ap()
    wq_d = nc.dram_tensor("wq", [F, M], F32R, kind="ExternalInput").ap()
    wk_d = nc.dram_tensor("wk", [F, M], F32R, kind="ExternalInput").ap()
    wv_d = nc.dram_tensor("wv", [F, M], F32R, kind="ExternalInput").ap()
    wo_d = nc.dram_tensor("wo", [M, F], F32R, kind="ExternalInput").ap()
    valid_d = nc.dram_tensor("valid", [128, KT, 1], F32, kind="ExternalInput").ap()
    out_d = nc.dram_tensor("out", [QL, F], F32, kind="ExternalOutput").ap()

    def run_units(queue, n_chains=2):
        """Round-robin-emit steps from `n_chains` units at a time.

        Alternating single steps between independent units keeps the
        in-order PE/ACT queues free of dependency stalls. A unit's
        prefetch() (DMA warm-up) is emitted one unit ahead of
        activation so chain switches don't expose DMA latency."""
        queue = list(queue)
        pf = 0
        started = 0

        def ensure_pf(upto):
            nonlocal pf
            while pf < min(upto, len(queue)):
                u = queue[pf]
                pf += 1
                if hasattr(u, "prefetch"):
                    u.prefetch()

        def take():
            nonlocal started
            if started < len(queue):
                ensure_pf(started + n_chains)
                g = queue[started].gen()
                started += 1
                return g
            return None

        active = []
        for _ in range(n_chains):
            g = take()
            if g:
                active.append(g)
        while active:
            nxt = []
            for g in active:
                try:
                    next(g)
                    nxt.append(g)
                except StopIteration:
                    g2 = take()
                    if g2:
                        nxt.append(g2)
            active = nxt

    with tile.TileContext(nc) as tc:
        with (
            tc.tile_pool(name="weights", bufs=1) as wp,
            tc.tile_pool(name="acts", bufs=1) as ap_,
            tc.tile_pool(name="ptp", bufs=3) as ptp,
            tc.tile_pool(name="small", bufs=2) as sp,
            tc.tile_pool(name="atp", bufs=2) as atp,
            tc.tile_pool(name="obp", bufs=2) as obp,
        ):
            # ---- resident weights (v first: its projection runs first) ----
            wv_s = wp.tile([128, 6, M], F32R)
            nc.sync.dma_start(out=wv_s, in_=wv_d.rearrange("(t p) m -> p t m", p=128))
            valid_s = wp.tile([128, KT, 1], F32)
            nc.sync.dma_start(out=valid_s, in_=valid_d)
            wk_s = wp.tile([128, 6, M], F32R)
            nc.sync.dma_start(out=wk_s, in_=wk_d.rearrange("(t p) m -> p t m", p=128))
            wq_s = wp.tile([128, 6, M], F32R)
            nc.sync.dma_start(out=wq_s, in_=wq_d.rearrange("(t p) m -> p t m", p=128))
            # ---- resident projected activations ----
            qhT = ap_.tile([128, 3, QL], F32R)       # feature-major, head pair p
            khT = ap_.tile([128, 3, KP], F32R)
            vh = ap_.tile([128, KT, H, D + 1], F32R)  # position-major + valid col

            # ---- stage A: projections (v position-major; k,q feature-major) --
            with (
                tc.tile_pool(name="xc", bufs=3) as xcp,
                tc.tile_pool(name="pp", bufs=4, space="PSUM") as ppp,
            ):
                class VUnit:
                    def __init__(self, c0):
                        self.c0 = c0
                        self.cw = min(QCW, KP - c0)

                    def prefetch(self):
                        self.xc = xcp.tile([128, 6, QCW], F32R, tag="xc",
                                           name=f"xcv{self.c0}")
                        nc.sync.dma_start(
                            out=self.xc[:, :, :self.cw],
                            in_=vT_d[:, self.c0:self.c0 + self.cw]
                            .rearrange("(t p) n -> p t n", p=128),
                        )

                    def gen(self):
                        for t in range(self.cw // 128):
                            kt = self.c0 // 128 + t
                            ps = ppp.tile([128, M], F32, tag="pp")
                            for K in range(6):
                                nc.tensor.matmul(
                                    ps,
                                    lhsT=self.xc[:, K, t * 128:(t + 1) * 128],
                                    rhs=wv_s[:, K, :],
                                    start=(K == 0), stop=(K == 5),
                                )
                            nc.vector.tensor_copy(
                                out=vh[:, kt, :, 0:D],
                                in_=ps.rearrange("p (h d) -> p h d", h=H),
                            )
                            yield

                class XUnit:
                    def __init__(self, xd, w_s, dst, c0, L):
                        self.xd, self.w_s, self.dst, self.c0 = xd, w_s, dst, c0
                        self.cw = min(QCW, L - c0)

                    def prefetch(self):
                        self.xc = xcp.tile([128, 6, QCW], F32R, tag="xc",
                                           name=f"xcx{id(self)}")
                        nc.sync.dma_start(
                            out=self.xc[:, :, :self.cw],
                            in_=self.xd[:, self.c0:self.c0 + self.cw]
                            .rearrange("(t p) n -> p t n", p=128),
                        )

                    def gen(self):
                        for m in range(3):
                            ps = ppp.tile([128, QCW], F32, tag="pp")
                            for K in range(6):
                                nc.tensor.matmul(
                                    ps[:, :self.cw],
                                    lhsT=self.w_s[:, K, m * 128:(m + 1) * 128],
                                    rhs=self.xc[:, K, :self.cw],
                                    start=(K == 0), stop=(K == 5),
                                )
                            nc.vector.tensor_copy(
                                out=self.dst[:, m, self.c0:self.c0 + self.cw],
                                in_=ps[:, :self.cw])
                            yield

                vq = [VUnit(c0) for c0 in range(0, KP, QCW)]
                kq = [XUnit(kT_d, wk_s, khT, c0, KP) for c0 in range(0, KP, QCW)]
                qq = [XUnit(qT_d, wq_s, qhT, c0, QL) for c0 in range(0, QL, QCW)]
                inter = []
                for i in range(max(len(vq), len(kq), len(qq))):
                    for li in (vq, kq, qq):
                        if i < len(li):
                            inter.append(li[i])
                for h in range(H):
                    nc.vector.tensor_copy(out=vh[:, :, h, D:D + 1], in_=valid_s)
                run_units(inter)

            wo_s = wp.tile([128, 3, F], F32R)
            nc.sync.dma_start(out=wo_s, in_=wo_d.rearrange("(t p) n -> p t n", p=128))

            # ---- stage B: attention + output projection ----
            with (
                tc.tile_pool(name="sps", bufs=2, space="PSUM") as sps,
                tc.tile_pool(name="acc", bufs=4, space="PSUM") as accp,
            ):
                at_tiles = {}

                class AttnUnit:
                    def __init__(self, qc, p):
                        self.qc, self.p = qc, p

                    def gen(self):
                        qc, p, q0 = self.qc, self.p, self.qc * QCW
                        if qc not in at_tiles:
                            at_tiles[qc] = atp.tile([128, 3, QCW], F32R, tag="at", name=f"at{qc}")
                        at = at_tiles[qc]
                        pa = accp.tile([D + 1, QCW], F32, tag="acc")
                        pb = accp.tile([D + 1, QCW], F32, tag="acc")
                        pacc = (pa, pb)
                        prev = None

                        def qk(kt):
                            ps = sps.tile([128, 2 * QCW], F32, tag="ps")
                            for j in range(2):
                                nc.tensor.matmul(
                                    ps[:, j * QCW:(j + 1) * QCW],
                                    lhsT=khT[j * 64:(j + 1) * 64, p,
                                             kt * 128:(kt + 1) * 128],
                                    rhs=qhT[j * 64:(j + 1) * 64, p, q0:q0 + QCW],
                                    start=True, stop=True,
                                )
                            pt = ptp.tile([128, 2 * QCW], F32R, tag="pt")
                            nc.scalar.activation(pt, ps, EXP, scale=0.125)
                            return pt

                        def pv(pt, kt):
                            for j in range(2):
                                nc.tensor.matmul(
                                    pacc[j],
                                    lhsT=vh[:, kt, 2 * p + j, :],
                                    rhs=pt[:, j * QCW:(j + 1) * QCW],
                                    start=(kt == 0), stop=(kt == KT - 1),
                                )

                        for kt in range(KT):
                            pt = qk(kt)
                            if prev is not None:
                                pv(*prev)
                            prev = (pt, kt)
                            yield
                        pv(*prev)
                        for j in range(2):
                            rr = sp.tile([1, QCW], F32, tag="rr")
                            nc.vector.reciprocal(rr, pacc[j][D:D + 1, :])
                            bc = sp.tile([64, QCW], F32, tag="bc")
                            nc.gpsimd.partition_broadcast(bc, rr)
                            nc.vector.tensor_mul(
                                at[j * 64:(j + 1) * 64, p, :],
                                pacc[j][0:D, :], bc)
                        yield

                class OprojUnit:
                    def __init__(self, qc):
                        self.qc = qc

                    def gen(self):
                        qc, q0 = self.qc, self.qc * QCW
                        at = at_tiles[qc]
                        for m2 in range(QCW // 128):
                            ob = obp.tile([128, F], F32, tag="ob")
                            for nch in range(2):
                                po = accp.tile([128, 384], F32, tag="acc")
                                for t in range(3):
                                    nc.tensor.matmul(
                                        po,
                                        lhsT=at[:, t, m2 * 128:(m2 + 1) * 128],
                                        rhs=wo_s[:, t, nch * 384:(nch + 1) * 384],
                                        start=(t == 0), stop=(t == 2),
                                    )
                                nc.vector.tensor_copy(
                                    out=ob[:, nch * 384:(nch + 1) * 384], in_=po)
                                yield
                            nc.sync.dma_start(
                                out=out_d[q0 + m2 * 128:q0 + (m2 + 1) * 128, :],
                                in_=ob,
                            )

                # Gated 2-chain interleave: an OprojUnit(qc) may only be
                # EMITTED after all three AttnUnits of qc finished emitting
                # (Tile program semantics follow emission order).
                attn_queue = [AttnUnit(qc, p) for qc in range(NQC)
                              for p in range(3)]
                remaining = {qc: 3 for qc in range(NQC)}
                ready_oproj = []
                attn_idx = 0

                def take_b():
                    nonlocal attn_idx
                    if ready_oproj:
                        u = ready_oproj.pop(0)
                        return (u.gen(), u)
                    if attn_idx < len(attn_queue):
                        u = attn_queue[attn_idx]
                        attn_idx += 1
                        return (u.gen(), u)
                    return None

                def finish_b(u):
                    if isinstance(u, AttnUnit):
                        remaining[u.qc] -= 1
                        if remaining[u.qc] == 0:
                            ready_oproj.append(OprojUnit(u.qc))

                active = []
                for _ in range(2):
                    t = take_b()
                    if t:
                        active.append(t)
                while active:
                    nxt = []
                    for g, u in active:
                        try:
                            next(g)
                            nxt.append((g, u))
                        except StopIteration:
                            finish_b(u)
                            t = take_b()
                            if t:
                                nxt.append(t)
                    active = nxt

    nc.compile()
    return nc


last_results = None


def _ensure_ntff_hook():
    """Install the axon NTFF profile hook if the image's antenv lacks it.

    trn_agent_boot intends to register this hook (see trn_boot.py step 6); on
    images whose antenv has no axon_hooks module it degrades. Recreate the
    module so trace=True works; silently no-op if anything is unavailable.
    """
    import sys
    import types
    try:
        import antenv.axon_hooks  # noqa: F401
        return
    except ImportError:
        pass
    try:
        import antenv
        from trn_agent_boot.trn_boot import _ntff_profile_via_ctypes
        hook = _ntff_profile_via_ctypes("/opt/axon/libaxon_pjrt.so")
        mod = types.ModuleType("antenv.axon_hooks")
        mod.get_axon_ntff_profile_hook = lambda: hook
        mod.set_axon_ntff_profile_hook = lambda h: None
        sys.modules["antenv.axon_hooks"] = mod
        antenv.axon_hooks = mod
    except Exception:
        pass


def kernel(v, k, q, mask, w_v, w_k, w_q, w_o, b_o):
    global last_results
    from concourse import bass_utils

    v = np.asarray(v, dtype=np.float32)
    k = np.asarray(k, dtype=np.float32)
    q = np.asarray(q, dtype=np.float32)
    mask = np.asarray(mask)
    w_v = np.asarray(w_v, dtype=np.float32)
    w_k = np.asarray(w_k, dtype=np.float32)
    w_q = np.asarray(w_q, dtype=np.float32)
    w_o = np.asarray(w_o, dtype=np.float32)
    b_o = np.asarray(b_o, dtype=np.float32)

    BZ, SL, F = q.shape
    QL = SL // 2

    kept = [np.flatnonzero(mask[b, 0, 0] == 0) for b in range(BZ)]
    klens = [len(x) for x in kept]
    KP = max(128, -(-max(klens) // 128) * 128)
    KT = KP // 128

    if KP not in _CACHE:
        _CACHE[KP] = _build(KP)
    nc = _CACHE[KP]

    # per-batch gathered/padded transposed k/v and validity tables
    kTs, vTs, valids = [], [], []
    for b in range(BZ):
        kt = np.zeros((F, KP), np.float32)
        vt = np.zeros((F, KP), np.float32)
        kt[:, :klens[b]] = k[b, kept[b]].T
        vt[:, :klens[b]] = v[b, kept[b]].T
        kTs.append(kt)
        vTs.append(vt)
        val = np.zeros(KP, np.float32)
        val[:klens[b]] = 1.0
        valids.append(np.ascontiguousarray(val.reshape(KT, 128).T)[:, :, None])

    in_maps = []
    for c in range(8):
        b, hg, qg = c // 4, (c // 2) % 2, c % 2
        mc = slice(384 * hg, 384 * (hg + 1))
        qr = slice(QL * qg, QL * (qg + 1))
        in_maps.append({
            "qT": np.ascontiguousarray(q[b, qr].T),
            "kT": kTs[b],
            "vT": vTs[b],
            "wq": np.ascontiguousarray(w_q[:, mc]),
            "wk": np.ascontiguousarray(w_k[:, mc]),
            "wv": np.ascontiguousarray(w_v[:, mc]),
            "wo": np.ascontiguousarray(w_o[mc, :]),
            "valid": valids[b],
        })

    trace = bool(os.environ.get("KERNEL_TRACE")) or bool(os.environ.get("BASS_TRACE"))
    if trace:
        _ensure_ntff_hook()
    try:
        res = bass_utils.run_bass_kernel_spmd(
            nc, in_maps, core_ids=list(range(8)),
            trace=trace,
            trace_cores=list(range(8)) if trace else None,
        )
    except Exception:
        if not trace:
            raise
        os.environ["BASS_NEVER_TRACE"] = "1"
        try:
            res = bass_utils.run_bass_kernel_spmd(
                nc, in_maps, core_ids=list(range(8)),
            )
        finally:
            del os.environ["BASS_NEVER_TRACE"]
    last_results = res

    out = np.empty((BZ, SL, F), np.float32)
    for b in range(BZ):
        for qg in range(2):
            c0 = b * 4 + qg          # hg = 0
            c1 = b * 4 + 2 + qg      # hg = 1
            out[b, QL * qg:QL * (qg + 1)] = (
                res.results[c0]["out"] + res.results[c1]["out"] + b_o
            )
    return out


# revision 16
# speedup vs baseline: 1.2951x; 1.0398x over previous
"""Multi-head attention (bz=2, slen=4096, d=768, 12 heads) on 8 trn2 NeuronCores.

Sharding: 8 cores = 2 (batch) x 2 (head halves of 6) x 2 (q halves of 2048).
Each core computes its q-slice of the partial output for its 6 heads; host sums
the two head-half partials per (batch, q-half) and adds b_o.

Key device-side structure (per core):
  - projections contract input features (768) on partitions; q/k inputs are fed
    pre-transposed [768, L] so qh^T/kh^T come out feature-major (QK operands),
    while vh is produced position-major (PV stationary operand) by using the
    transposed v as the stationary operand instead.
  - mask handling is exact and free: masked k-positions are gathered away on the
    host (softmax with -1e9 gives exactly 0 in fp32), and right-padding to a
    multiple of 128 is neutralized by a per-position "valid" column appended to
    v, which simultaneously produces the softmax row-sums during PV.
  - scores stay in S^T layout [k-pos partitions, q free]: softmax needs only one
    ACT pass (exp with fused 1/sqrt(64) scale); row-sums fall out of PV; the
    per-(head,q) normalization is applied to attn^T (64 rows) instead of P.
  - QK packs head pairs into the PE array rows (K=64 x 2 via tile_position).
  - matmuls run in float32r (full fp32 data, fast PE mode).
"""

import os

import numpy as np

_CACHE = {}


def _build(KP):
    import concourse.bass as bass
    import concourse.mybir as mybir
    import concourse.tile as tile
    from concourse import bacc

    F32 = mybir.dt.float32
    F32R = mybir.dt.float32r
    EXP = mybir.ActivationFunctionType.Exp

    F = 768          # model dim
    M = 384          # output features per core (6 heads x 64)
    QL = 2048        # q rows per core
    D = 64           # head dim
    H = 6            # heads per core
    KT = KP // 128   # k tiles
    QCW = 512        # q chunk width
    NQC = QL // QCW

    nc = bacc.Bacc("TRN2", target_bir_lowering=False, debug=False, num_devices=8)

    qT_d = nc.dram_tensor("qT", [F, QL], F32R, kind="ExternalInput").ap()
    kT_d = nc.dram_tensor("kT", [F, KP], F32R, kind="ExternalInput").ap()
    vT_d = nc.dram_tensor("vT", [F, KP], F32R, kind="ExternalInput").ap()
    wq_d = nc.dram_tensor("wq", [F, M], F32R, kind="ExternalInput").ap()
    wk_d = nc.dram_tensor("wk", [F, M], F32R, kind="ExternalInput").ap()
    wv_d = nc.dram_tensor("wv", [F, M], F32R, kind="ExternalInput").ap()
    wo_d = nc.dram_tensor("wo", [M, F], F32R, kind="ExternalInput").ap()
    valid_d = nc.dram_tensor("valid", [128, KT, 1], F32, kind="ExternalInput").ap()
    out_d = nc.dram_tensor("out", [QL, F], F32, kind="ExternalOutput").ap()

    def run_units(queue, n_chains=2):
        """Round-robin-emit steps from `n_chains` units at a time.

        Alternating single steps between independent units keeps the
        in-order PE/ACT queues free of dependency stalls. A unit's
        prefetch() (DMA warm-up) is emitted one unit ahead of
        activation so chain switches don't expose DMA latency."""
        queue = list(queue)
        pf = 0
        started = 0

        def ensure_pf(upto):
            nonlocal pf
            while pf < min(upto, len(queue)):
                u = queue[pf]
                pf += 1
                if hasattr(u, "prefetch"):
                    u.prefetch()

        def take():
            nonlocal started
            if started < len(queue):
                ensure_pf(started + n_chains)
                g = queue[started].gen()
                started += 1
                return g
            return None

        active = []
        for _ in range(n_chains):
            g = take()
            if g:
                active.append(g)
        while active:
            nxt = []
            for g in active:
                try:
                    next(g)
                    nxt.append(g)
                except StopIteration:
                    g2 = take()
                    if g2:
                        nxt.append(g2)
            active = nxt

    with tile.TileContext(nc) as tc:
        with (
            tc.tile_pool(name="weights", bufs=1) as wp,
            tc.tile_pool(name="acts", bufs=1) as ap_,
            tc.tile_pool(name="ptp", bufs=3) as ptp,
            tc.tile_pool(name="small", bufs=2) as sp,
            tc.tile_pool(name="atp", bufs=2) as atp,
            tc.tile_pool(name="obp", bufs=2) as obp,
        ):
            # ---- resident weights (v first: its projection runs first) ----
            wv_s = wp.tile([128, 6, M], F32R)
            nc.sync.dma_start(out=wv_s, in_=wv_d.rearrange("(t p) m -> p t m", p=128))
            valid_s = wp.tile([128, KT, 1], F32)
            nc.sync.dma_start(out=valid_s, in_=valid_d)
            wk_s = wp.tile([128, 6, M], F32R)
            wq_s = wp.tile([128, 6, M], F32R)

            def dma_wk():
                nc.sync.dma_start(
                    out=wk_s, in_=wk_d.rearrange("(t p) m -> p t m", p=128))

            def dma_wq():
                nc.sync.dma_start(
                    out=wq_s, in_=wq_d.rearrange("(t p) m -> p t m", p=128))
            # ---- resident projected activations ----
            qhT = ap_.tile([128, 3, QL], F32R)       # feature-major, head pair p
            khT = ap_.tile([128, 3, KP], F32R)
            vh = ap_.tile([128, KT, H, D + 1], F32R)  # position-major + valid col

            xcp_cm = tc.tile_pool(name="xc", bufs=3)
            xcp = xcp_cm.__enter__()

            # ---- stage A: projections (v position-major; k feature-major) ----
            with (
                tc.tile_pool(name="pp", bufs=4, space="PSUM") as ppp,
            ):
                class VUnit:
                    def __init__(self, c0):
                        self.c0 = c0
                        self.cw = min(QCW, KP - c0)

                    def prefetch(self):
                        self.xc = xcp.tile([128, 6, QCW], F32R, tag="xc",
                                           name=f"xcv{self.c0}")
                        nc.sync.dma_start(
                            out=self.xc[:, :, :self.cw],
                            in_=vT_d[:, self.c0:self.c0 + self.cw]
                            .rearrange("(t p) n -> p t n", p=128),
                        )

                    def gen(self):
                        for t in range(self.cw // 128):
                            kt = self.c0 // 128 + t
                            ps = ppp.tile([128, M], F32, tag="pp")
                            for K in range(6):
                                nc.tensor.matmul(
                                    ps,
                                    lhsT=self.xc[:, K, t * 128:(t + 1) * 128],
                                    rhs=wv_s[:, K, :],
                                    start=(K == 0), stop=(K == 5),
                                )
                            nc.vector.tensor_copy(
                                out=vh[:, kt, :, 0:D],
                                in_=ps.rearrange("p (h d) -> p h d", h=H),
                            )
                            yield

                class XUnit:
                    def __init__(self, xd, w_s, dst, c0, L, pool, tag,
                                 wdma=None):
                        self.xd, self.w_s, self.dst, self.c0 = xd, w_s, dst, c0
                        self.cw = min(QCW, L - c0)
                        self.pool, self.tag, self.wdma = pool, tag, wdma

                    def prefetch(self):
                        if self.wdma is not None:
                            self.wdma()
                        self.xc = xcp.tile([128, 6, QCW], F32R, tag="xc",
                                           name=f"xcx{id(self)}")
                        nc.sync.dma_start(
                            out=self.xc[:, :, :self.cw],
                            in_=self.xd[:, self.c0:self.c0 + self.cw]
                            .rearrange("(t p) n -> p t n", p=128),
                        )

                    def gen(self):
                        for m in range(3):
                            ps = self.pool.tile([128, QCW], F32, tag=self.tag)
                            for K in range(6):
                                nc.tensor.matmul(
                                    ps[:, :self.cw],
                                    lhsT=self.w_s[:, K, m * 128:(m + 1) * 128],
                                    rhs=self.xc[:, K, :self.cw],
                                    start=(K == 0), stop=(K == 5),
                                )
                            nc.vector.tensor_copy(
                                out=self.dst[:, m, self.c0:self.c0 + self.cw],
                                in_=ps[:, :self.cw])
                            yield

                vq = [VUnit(c0) for c0 in range(0, KP, QCW)]
                kq = [XUnit(kT_d, wk_s, khT, c0, KP, ppp, "pp",
                            wdma=dma_wk if c0 == 0 else None)
                      for c0 in range(0, KP, QCW)]
                inter = []
                for i in range(max(len(vq), len(kq))):
                    for li in (vq, kq):
                        if i < len(li):
                            inter.append(li[i])
                for h in range(H):
                    nc.vector.tensor_copy(out=vh[:, :, h, D:D + 1], in_=valid_s)
                run_units(inter)

            wo_s = wp.tile([128, 3, F], F32R)
            nc.sync.dma_start(out=wo_s, in_=wo_d.rearrange("(t p) n -> p t n", p=128))

            # ---- stage B: attention + output projection ----
            with (
                tc.tile_pool(name="sps", bufs=2, space="PSUM") as sps,
                tc.tile_pool(name="acc", bufs=4, space="PSUM") as accp,
            ):
                at_tiles = {}

                class AttnUnit:
                    def __init__(self, qc, p):
                        self.qc, self.p = qc, p

                    def gen(self):
                        qc, p, q0 = self.qc, self.p, self.qc * QCW
                        if qc not in at_tiles:
                            at_tiles[qc] = atp.tile([128, 3, QCW], F32R, tag="at", name=f"at{qc}")
                        at = at_tiles[qc]
                        pa = accp.tile([D + 1, QCW], F32, tag="acc")
                        pb = accp.tile([D + 1, QCW], F32, tag="acc")
                        pacc = (pa, pb)
                        prev = None

                        def qk(kt):
                            ps = sps.tile([128, 2 * QCW], F32, tag="ps")
                            for j in range(2):
                                nc.tensor.matmul(
                                    ps[:, j * QCW:(j + 1) * QCW],
                                    lhsT=khT[j * 64:(j + 1) * 64, p,
                                             kt * 128:(kt + 1) * 128],
                                    rhs=qhT[j * 64:(j + 1) * 64, p, q0:q0 + QCW],
                                    start=True, stop=True,
                                )
                            pt = ptp.tile([128, 2 * QCW], F32R, tag="pt")
                            nc.scalar.activation(pt, ps, EXP, scale=0.125)
                            return pt

                        def pv(pt, kt):
                            for j in range(2):
                                nc.tensor.matmul(
                                    pacc[j],
                                    lhsT=vh[:, kt, 2 * p + j, :],
                                    rhs=pt[:, j * QCW:(j + 1) * QCW],
                                    start=(kt == 0), stop=(kt == KT - 1),
                                )

                        for kt in range(KT):
                            pt = qk(kt)
                            if prev is not None:
                                pv(*prev)
                            prev = (pt, kt)
                            yield
                        pv(*prev)
                        for j in range(2):
                            rr = sp.tile([1, QCW], F32, tag="rr")
                            nc.vector.reciprocal(rr, pacc[j][D:D + 1, :])
                            bc = sp.tile([64, QCW], F32, tag="bc")
                            nc.gpsimd.partition_broadcast(bc, rr)
                            nc.vector.tensor_mul(
                                at[j * 64:(j + 1) * 64, p, :],
                                pacc[j][0:D, :], bc)
                        yield

                class OprojUnit:
                    def __init__(self, qc):
                        self.qc = qc

                    def gen(self):
                        qc, q0 = self.qc, self.qc * QCW
                        at = at_tiles[qc]
                        for m2 in range(QCW // 128):
                            ob = obp.tile([128, F], F32, tag="ob")
                            for nch in range(2):
                                po = accp.tile([128, 384], F32, tag="acc")
                                for t in range(3):
                                    nc.tensor.matmul(
                                        po,
                                        lhsT=at[:, t, m2 * 128:(m2 + 1) * 128],
                                        rhs=wo_s[:, t, nch * 384:(nch + 1) * 384],
                                        start=(t == 0), stop=(t == 2),
                                    )
                                nc.vector.tensor_copy(
                                    out=ob[:, nch * 384:(nch + 1) * 384], in_=po)
                                yield
                            nc.sync.dma_start(
                                out=out_d[q0 + m2 * 128:q0 + (m2 + 1) * 128, :],
                                in_=ob,
                            )

                # Gated 2-chain interleave: an OprojUnit(qc) may only be
                # EMITTED after all three AttnUnits of qc finished emitting
                # (Tile program semantics follow emission order). The
                # q-projection chunk units ride the same scheduler so they
                # overlap attention instead of serializing stage A.
                qunits = [XUnit(qT_d, wq_s, qhT, c * QCW, QL, accp, "acc",
                                wdma=dma_wq if c == 0 else None)
                          for c in range(NQC)]
                attn_queue = [qunits[0], qunits[1]]
                for qc in range(NQC):
                    for p in range(3):
                        attn_queue.append(AttnUnit(qc, p))
                    if qc + 2 < NQC:
                        attn_queue.append(qunits[qc + 2])
                remaining = {qc: 3 for qc in range(NQC)}
                ready_oproj = []
                attn_idx = 0
                pf_idx = 0

                def ensure_pf_b(upto):
                    nonlocal pf_idx
                    while pf_idx < min(upto, len(attn_queue)):
                        u = attn_queue[pf_idx]
                        pf_idx += 1
                        if hasattr(u, "prefetch"):
                            u.prefetch()

                def take_b():
                    nonlocal attn_idx
                    if ready_oproj:
                        u = ready_oproj.pop(0)
                        return (u.gen(), u)
                    if attn_idx < len(attn_queue):
                        u = attn_queue[attn_idx]
                        attn_idx += 1
                        ensure_pf_b(attn_idx + 2)
                        return (u.gen(), u)
                    return None

                def finish_b(u):
                    if isinstance(u, AttnUnit):
                        remaining[u.qc] -= 1
                        if remaining[u.qc] == 0:
                            ready_oproj.append(OprojUnit(u.qc))

                active = []
                for _ in range(2):
                    t = take_b()
                    if t:
                        active.append(t)
                while active:
                    nxt = []
                    for g, u in active:
                        try:
                            next(g)
                            nxt.append((g, u))
                        except StopIteration:
                            finish_b(u)
                            t = take_b()
                            if t:
                                nxt.append(t)
                    active = nxt

            xcp_cm.__exit__(None, None, None)

    nc.compile()
    return nc


last_results = None


def _ensure_ntff_hook():
    """Install the axon NTFF profile hook if the image's antenv lacks it.

    trn_agent_boot intends to register this hook (see trn_boot.py step 6); on
    images whose antenv has no axon_hooks module it degrades. Recreate the
    module so trace=True works; silently no-op if anything is unavailable.
    """
    import sys
    import types
    try:
        import antenv.axon_hooks  # noqa: F401
        return
    except ImportError:
        pass
    try:
        import antenv
        from trn_agent_boot.trn_boot import _ntff_profile_via_ctypes
        hook = _ntff_profile_via_ctypes("/opt/axon/libaxon_pjrt.so")
        mod = types.ModuleType("antenv.axon_hooks")
        mod.get_axon_ntff_profile_hook = lambda: hook
        mod.set_axon_ntff_profile_hook = lambda h: None
        sys.modules["antenv.axon_hooks"] = mod
        antenv.axon_hooks = mod
    except Exception:
        pass


def kernel(v, k, q, mask, w_v, w_k, w_q, w_o, b_o):
    global last_results
    from concourse import bass_utils

    v = np.asarray(v, dtype=np.float32)
    k = np.asarray(k, dtype=np.float32)
    q = np.asarray(q, dtype=np.float32)
    mask = np.asarray(mask)
    w_v = np.asarray(w_v, dtype=np.float32)
    w_k = np.asarray(w_k, dtype=np.float32)
    w_q = np.asarray(w_q, dtype=np.float32)
    w_o = np.asarray(w_o, dtype=np.float32)
    b_o = np.asarray(b_o, dtype=np.float32)

    BZ, SL, F = q.shape
    QL = SL // 2

    kept = [np.flatnonzero(mask[b, 0, 0] == 0) for b in range(BZ)]
    klens = [len(x) for x in kept]
    KP = max(128, -(-max(klens) // 128) * 128)
    KT = KP // 128

    if KP not in _CACHE:
        _CACHE[KP] = _build(KP)
    nc = _CACHE[KP]

    # per-batch gathered/padded transposed k/v and validity tables
    kTs, vTs, valids = [], [], []
    for b in range(BZ):
        kt = np.zeros((F, KP), np.float32)
        vt = np.zeros((F, KP), np.float32)
        kt[:, :klens[b]] = k[b, kept[b]].T
        vt[:, :klens[b]] = v[b, kept[b]].T
        kTs.append(kt)
        vTs.append(vt)
        val = np.zeros(KP, np.float32)
        val[:klens[b]] = 1.0
        valids.append(np.ascontiguousarray(val.reshape(KT, 128).T)[:, :, None])

    in_maps = []
    for c in range(8):
        b, hg, qg = c // 4, (c // 2) % 2, c % 2
        mc = slice(384 * hg, 384 * (hg + 1))
        qr = slice(QL * qg, QL * (qg + 1))
        in_maps.append({
            "qT": np.ascontiguousarray(q[b, qr].T),
            "kT": kTs[b],
            "vT": vTs[b],
            "wq": np.ascontiguousarray(w_q[:, mc]),
            "wk": np.ascontiguousarray(w_k[:, mc]),
            "wv": np.ascontiguousarray(w_v[:, mc]),
            "wo": np.ascontiguousarray(w_o[mc, :]),
            "valid": valids[b],
        })

    trace = bool(os.environ.get("KERNEL_TRACE")) or bool(os.environ.get("BASS_TRACE"))
    if trace:
        _ensure_ntff_hook()
    tcores = os.environ.get("KERNEL_TRACE_CORES", "0,1,2,3,4,5,6,7")
    tcores = [int(x) for x in tcores.split(",")]
    try:
        res = bass_utils.run_bass_kernel_spmd(
            nc, in_maps, core_ids=list(range(8)),
            trace=trace,
            trace_cores=tcores if trace else None,
        )
    except Exception:
        if not trace:
            raise
        os.environ["BASS_NEVER_TRACE"] = "1"
        try:
            res = bass_utils.run_bass_kernel_spmd(
                nc, in_maps, core_ids=list(range(8)),
            )
        finally:
            del os.environ["BASS_NEVER_TRACE"]
    last_results = res

    out = np.empty((BZ, SL, F), np.float32)
    for b in range(BZ):
        for qg in range(2):
            c0 = b * 4 + qg          # hg = 0
            c1 = b * 4 + 2 + qg      # hg = 1
            out[b, QL * qg:QL * (qg + 1)] = (
                res.results[c0]["out"] + res.results[c1]["out"] + b_o
            )
    return out
